# revision 1
# baseline (speedup 1.0000x reference)
"""Trainium2 Bass/Tile kernel for nn_CAVAModule (cross-attention A/V alignment).

Math notes (exact simplifications of the reference):
  - delta = 2 + 4*sigmoid(clip(theta,-12,12)) is in [2, 6], so the mask
    center min(max(t+delta,0),t) == t for every t: the displacement-aware
    causal mask is a fixed 6-tap causal moving average, independent of theta.
  - The soft temporal shift (2-tap linear interp at integer lag n=floor(delta))
    composed with that moving average is a banded Toeplitz operator over time
    with a <=8-tap band. Only rows t < 12 (clipping at t=0) deviate from the
    Toeplitz pattern, so the whole shift+mask+einsum collapses into three
    128x128 blocks: C00 (tile 0), Csub/Cdiag (every later tile), applied as
    PE matmuls against the 128-token LayerNorm'd audio tiles.
  - ACT table discipline: Gelu/Tanh/Square/Copy/Identity share one table set;
    sigmoid(l) = 0.5 + 0.5*tanh(l/2) keeps the gate in that set, and Sqrt is
    batched NB tiles at a time to amortize its 1.3us table switch.
  - l2_normalize(LN(x)): the LN scale rstd cancels inside the l2 norm, so
    vn = w/||w|| with w = x-mu; rstd_v is folded into the final (1-g)*v term.

Sharding: data-parallel over batch, one sample per NeuronCore, no cross-core
communication. 16 tiles of 128 tokens, processed in batches of NB=4.

Matmuls run as float32r (TF32-like fast PE path); the gate-only path
(vn/an -> transposes -> x -> W1 -> h) runs bf16 -- sigmoid saturation and the
[0.05,0.95] clip absorb the logit noise, so end-to-end error stays ~1.6e-4.
"""

import sys

for _p in ("/opt/trn_rl_repo",):
    if _p not in sys.path:
        sys.path.insert(0, _p)

import ml_dtypes
import numpy as np

import concourse.bacc as bacc
import concourse.bass as bass
import concourse.tile as tile
from concourse import mybir
from concourse.bass_utils import run_bass_kernel_spmd

F32 = mybir.dt.float32
F32R = mybir.dt.float32r
BF16 = mybir.dt.bfloat16
ALU = mybir.AluOpType
ACT = mybir.ActivationFunctionType

B, T, VDIM, ADIM, DM = 8, 2048, 1024, 768, 256
HID = 1024
P = 128
NT = T // P          # 16 token tiles
KV = VDIM // P       # 8
KA = ADIM // P       # 6
KX = (3 * DM) // P   # 6
NB = 4               # token tiles per sqrt-batch (and per input DMA chunk)
LN_EPS = 1e-5
L2_EPS = 1e-8
WIN = 6              # mask window taps (tau in [t-5, t])
RS2 = float(1.0 / np.sqrt(2.0))

_nc_cache: dict = {}


def _build_cmats(delta: float) -> np.ndarray:
    """Three [tau, t] blocks of the combined shift+mask operator."""
    dl = min(max(delta, 0.0), float(T - 1))
    n = int(np.floor(dl))
    alpha = dl - n

    def row_w(t):
        w = np.zeros(2 * P, np.float64)
        m = min(t + 1, WIN)
        for s in range(max(0, t - (WIN - 1)), t + 1):
            i0 = min(max(s - n, 0), T - 1)
            i1 = min(i0 + 1, T - 1)
            w[i0] += (1.0 - alpha) / m
            w[i1] += alpha / m
        return w

    c00 = np.zeros((P, P), np.float64)
    csub = np.zeros((P, P), np.float64)
    cdiag = np.zeros((P, P), np.float64)
    for t in range(P):
        w = row_w(t)
        c00[:, t] = w[:P]
        w = row_w(P + t)
        csub[:, t] = w[:P]
        cdiag[:, t] = w[P:2 * P]
    return np.ascontiguousarray(np.stack([c00, csub, cdiag]).astype(np.float32))


def _build(bv_nz: bool, ba_nz: bool, b1_nz: bool, b2f: float):
    nc = bacc.Bacc("TRN2", target_bir_lowering=False, debug=False, num_devices=8)

    # f32r for tensors consumed only by matmuls (PE rounds on read)
    vT = nc.dram_tensor("vT", [VDIM, T], F32R, kind="ExternalInput")
    aT = nc.dram_tensor("aT", [ADIM, T], F32R, kind="ExternalInput")
    wv = nc.dram_tensor("wv", [VDIM, DM], F32R, kind="ExternalInput")
    wa = nc.dram_tensor("wa", [ADIM, DM], F32R, kind="ExternalInput")
    w1 = nc.dram_tensor("w1", [3 * DM, HID], BF16, kind="ExternalInput")
    w2h = nc.dram_tensor("w2h", [HID], F32, kind="ExternalInput")
    cm = nc.dram_tensor("cm", [3, P, P], F32R, kind="ExternalInput")
    ident = nc.dram_tensor("ident", [P, P], BF16, kind="ExternalInput")
    if bv_nz:
        bvr = nc.dram_tensor("bvr", [1, DM], F32R, kind="ExternalInput")
    if ba_nz:
        bar = nc.dram_tensor("bar", [1, DM], F32R, kind="ExternalInput")
    if b1_nz:
        b1r = nc.dram_tensor("b1r", [1, HID], F32R, kind="ExternalInput")
    out = nc.dram_tensor("out", [T, DM], F32, kind="ExternalOutput")

    def bcast(handle_ap, n):
        return bass.AP(
            tensor=handle_ap.tensor, offset=handle_ap.offset, ap=[[0, P], [1, n]]
        )

    with tile.TileContext(nc) as tc:
        with (
            tc.tile_pool(name="singles", bufs=1) as singles,
            tc.tile_pool(name="vchunk", bufs=2) as vchunk,
            tc.tile_pool(name="achunk", bufs=2) as achunk,
            tc.tile_pool(name="wvp", bufs=NB + 2) as wv_pool,     # centered video
            tc.tile_pool(name="wap", bufs=NB + 1) as wa_pool,     # centered audio
            tc.tile_pool(name="ap", bufs=3) as a_pool,            # LN'd audio (f32r)
            tc.tile_pool(name="actxp", bufs=NB + 2) as actx_pool,
            tc.tile_pool(name="scr", bufs=2) as scratch,
            tc.tile_pool(name="xtp", bufs=2) as xt_pool,
            tc.tile_pool(name="hb", bufs=2) as hbuf,
            tc.tile_pool(name="ob", bufs=3) as obuf,
            tc.tile_pool(name="small", bufs=10) as small,
            tc.tile_pool(name="bsm", bufs=2) as batch_small,
            tc.tile_pool(name="psum_mm", bufs=4, space="PSUM") as psum_mm,
            tc.tile_pool(name="psum_ct", bufs=2, space="PSUM") as psum_ct,
            tc.tile_pool(name="psum_h", bufs=1, space="PSUM") as psum_h,
        ):
            # ---- persistent weights/constants ----
            wv_sb = singles.tile([P, KV, DM], F32R)
            nc.gpsimd.dma_start(out=wv_sb, in_=wv.ap().rearrange("(ko p) n -> p ko n", p=P))
            wa_sb = singles.tile([P, KA, DM], F32R)
            nc.gpsimd.dma_start(out=wa_sb, in_=wa.ap().rearrange("(ko p) n -> p ko n", p=P))
            w1_sb = singles.tile([P, KX, HID], BF16)
            nc.gpsimd.dma_start(out=w1_sb, in_=w1.ap().rearrange("(ko p) n -> p ko n", p=P))
            w2_sb = singles.tile([P, HID], F32)
            nc.gpsimd.dma_start(out=w2_sb, in_=bcast(w2h.ap(), HID))
            cm_sb = singles.tile([P, 3, P], F32R)
            nc.gpsimd.dma_start(out=cm_sb, in_=cm.ap().rearrange("c p t -> p c t"))
            id_sb = singles.tile([P, P], BF16)
            nc.gpsimd.dma_start(out=id_sb, in_=ident.ap())
            eps_sb = singles.tile([P, 1], F32)
            nc.vector.memset(eps_sb, LN_EPS)
            if bv_nz or ba_nz or b1_nz:
                ones_sb = singles.tile([1, P], F32R)
                nc.vector.memset(ones_sb, 1.0)
            if bv_nz:
                bv_sb = singles.tile([1, DM], F32R)
                nc.sync.dma_start(out=bv_sb, in_=bvr.ap())
            if ba_nz:
                ba_sb = singles.tile([1, DM], F32R)
                nc.sync.dma_start(out=ba_sb, in_=bar.ap())
            if b1_nz:
                b1_sb = singles.tile([1, HID], F32R)
                nc.sync.dma_start(out=b1_sb, in_=b1r.ap())

            vT_r = vT.ap().rearrange("(ko p) t -> p ko t", p=P)
            aT_r = aT.ap().rearrange("(ko p) t -> p ko t", p=P)

            a_prev = None
            for bidx in range(NT // NB):
                # input chunk for this batch (NB tiles of 128 tokens)
                vt_sb = vchunk.tile([P, KV, NB * P], F32R)
                nc.sync.dma_start(
                    out=vt_sb, in_=vT_r[:, :, bidx * NB * P:(bidx + 1) * NB * P])
                at_sb = achunk.tile([P, KA, NB * P], F32R)
                nc.sync.dma_start(
                    out=at_sb, in_=aT_r[:, :, bidx * NB * P:(bidx + 1) * NB * P])

                # batch stats tiles
                mv_all = batch_small.tile([P, 2, NB, 2], F32, tag="mv")
                std_all = batch_small.tile([P, 2, NB], F32, tag="std")
                rstd_all = batch_small.tile([P, 2, NB], F32, tag="rstd")
                ssq_all = batch_small.tile([P, 2, NB], F32, tag="ssq")
                nrm_all = batch_small.tile([P, 2, NB], F32, tag="nrm")
                rn_all = batch_small.tile([P, 2, NB], F32, tag="rn")

                wv_t, wa_t = [], []
                # ---- phase A: projections + stats + centering ----
                for j in range(NB):
                    tsl = slice(j * P, (j + 1) * P)
                    pv = psum_mm.tile([P, DM], F32, tag="mm")
                    for k in range(KV):
                        nc.tensor.matmul(pv, lhsT=vt_sb[:, k, tsl],
                                         rhs=wv_sb[:, k, :],
                                         start=(k == 0),
                                         stop=(k == KV - 1 and not bv_nz))
                    if bv_nz:
                        nc.tensor.matmul(pv, lhsT=ones_sb, rhs=bv_sb,
                                         start=False, stop=True)
                    stats = small.tile([P, nc.vector.BN_STATS_DIM], F32, tag="st")
                    nc.vector.bn_stats(out=stats, in_=pv)
                    nc.vector.bn_aggr(out=mv_all[:, 0, j, :], in_=stats)

                    pa = psum_mm.tile([P, DM], F32, tag="mm")
                    for k in range(KA):
                        nc.tensor.matmul(pa, lhsT=at_sb[:, k, tsl],
                                         rhs=wa_sb[:, k, :],
                                         start=(k == 0),
                                         stop=(k == KA - 1 and not ba_nz))
                    if ba_nz:
                        nc.tensor.matmul(pa, lhsT=ones_sb, rhs=ba_sb,
                                         start=False, stop=True)
                    stats2 = small.tile([P, nc.vector.BN_STATS_DIM], F32, tag="st")
                    nc.vector.bn_stats(out=stats2, in_=pa)
                    nc.vector.bn_aggr(out=mv_all[:, 1, j, :], in_=stats2)

                    w_v = wv_pool.tile([P, DM], F32)
                    nc.vector.tensor_scalar(out=w_v, in0=pv,
                                            scalar1=mv_all[:, 0, j, 0:1],
                                            scalar2=None, op0=ALU.subtract)
                    w_a = wa_pool.tile([P, DM], F32)
                    nc.vector.tensor_scalar(out=w_a, in0=pa,
                                            scalar1=mv_all[:, 1, j, 0:1],
                                            scalar2=None, op0=ALU.subtract)
                    wv_t.append(w_v)
                    wa_t.append(w_a)
                    # ||w_v||^2 while we're here (DVE, with accumulator)
                    sqs = scratch.tile([P, DM], F32, tag="sq")
                    nc.vector.scalar_tensor_tensor(
                        out=sqs, in0=w_v, scalar=1.0, in1=w_v,
                        op0=ALU.mult, op1=ALU.mult,
                        accum_out=ssq_all[:, 0, j:j + 1])

                # ---- batched sqrt #1: rstd for v and a ----
                nc.scalar.activation(out=std_all[:, :, :, None],
                                     in_=mv_all[:, :, :, 1:2], func=ACT.Sqrt,
                                     bias=eps_sb, scale=1.0)
                nc.vector.reciprocal(out=rstd_all, in_=std_all)
                nrstd = batch_small.tile([P, 2, NB], F32, tag="nrstd")
                nc.vector.tensor_scalar(out=nrstd, in0=rstd_all, scalar1=-1.0,
                                        scalar2=None, op0=ALU.mult)

                # ---- phase B: audio LN scale, context matmul, ctx norms ----
                actx_t = []
                for j in range(NB):
                    a_sb = a_pool.tile([P, DM], F32R)
                    nc.scalar.activation(out=a_sb, in_=wa_t[j], func=ACT.Copy,
                                         scale=rstd_all[:, 1, j:j + 1])
                    pc = psum_ct.tile([P, DM], F32, tag="ct")
                    if bidx == 0 and j == 0:
                        nc.tensor.matmul(pc, lhsT=cm_sb[:, 0, :], rhs=a_sb,
                                         start=True, stop=True)
                    else:
                        nc.tensor.matmul(pc, lhsT=cm_sb[:, 1, :], rhs=a_prev,
                                         start=True, stop=False)
                        nc.tensor.matmul(pc, lhsT=cm_sb[:, 2, :], rhs=a_sb,
                                         start=False, stop=True)
                    a_prev = a_sb
                    # ||a_ctx||^2 (ACT Square with accumulator) + copy to SBUF
                    sqa = scratch.tile([P, DM], F32, tag="sq")
                    nc.scalar.activation(out=sqa, in_=pc, func=ACT.Square,
                                         accum_out=ssq_all[:, 1, j:j + 1])
                    actx = actx_pool.tile([P, DM], F32)
                    nc.scalar.copy(out=actx, in_=pc)
                    actx_t.append(actx)

                # ---- batched sqrt #2: 1/max(||.||, eps) ----
                nc.scalar.activation(out=nrm_all, in_=ssq_all, func=ACT.Sqrt)
                nc.vector.tensor_scalar_max(out=nrm_all, in0=nrm_all, scalar1=L2_EPS)
                nc.vector.reciprocal(out=rn_all, in_=nrm_all)

                # ---- phase C: gate MLP + fuse ----
                for j in range(NB):
                    w_v = wv_t[j]
                    actx = actx_t[j]
                    vn = scratch.tile([P, DM], BF16, tag="vn")
                    nc.scalar.activation(out=vn, in_=w_v, func=ACT.Copy,
                                         scale=rn_all[:, 0, j:j + 1])
                    an = scratch.tile([P, DM], BF16, tag="an")
                    nc.scalar.activation(out=an, in_=actx, func=ACT.Copy,
                                         scale=rn_all[:, 1, j:j + 1])

                    pt = psum_ct.tile([P, 4, P], BF16, tag="ct")
                    for k in range(2):
                        nc.tensor.transpose(pt[:, k, :], an[:, k * P:(k + 1) * P], id_sb)
                    for k in range(2):
                        nc.tensor.transpose(pt[:, 2 + k, :], vn[:, k * P:(k + 1) * P], id_sb)
                    xt = xt_pool.tile([P, KX, P], BF16)
                    nc.vector.tensor_copy(out=xt[:, 0:4, :], in_=pt)
                    nc.gpsimd.tensor_mul(out=xt[:, 4:6, :], in0=xt[:, 0:2, :],
                                         in1=xt[:, 2:4, :])

                    ph = psum_h.tile([P, HID], F32)
                    for nh in range(2):
                        psl = ph[:, nh * 512:(nh + 1) * 512]
                        for k in range(KX):
                            nc.tensor.matmul(
                                psl, lhsT=xt[:, k, :],
                                rhs=w1_sb[:, k, nh * 512:(nh + 1) * 512],
                                start=(k == 0), stop=(k == KX - 1 and not b1_nz))
                        if b1_nz:
                            nc.tensor.matmul(psl, lhsT=ones_sb,
                                             rhs=b1_sb[:, nh * 512:(nh + 1) * 512],
                                             start=False, stop=True)
                    hh = hbuf.tile([P, HID], F32, tag="hh")
                    for nh in range(2):
                        sl = slice(nh * 512, (nh + 1) * 512)
                        nc.scalar.activation(out=hh[:, sl], in_=ph[:, sl],
                                             func=ACT.Gelu)

                    lsc = hbuf.tile([P, HID], F32, tag="lsc")
                    lg = small.tile([P, 1], F32, tag="lg")
                    nc.vector.scalar_tensor_tensor(out=lsc, in0=hh, scalar=0.0,
                                                   in1=w2_sb, op0=ALU.bypass,
                                                   op1=ALU.mult, accum_out=lg)
                    if b2f != 0.0:
                        nc.vector.tensor_scalar(out=lg, in0=lg, scalar1=float(b2f),
                                                scalar2=-12.0, op0=ALU.add,
                                                op1=ALU.max)
                        nc.vector.tensor_scalar(out=lg, in0=lg, scalar1=12.0,
                                                scalar2=None, op0=ALU.min)
                    else:
                        nc.vector.tensor_scalar(out=lg, in0=lg, scalar1=-12.0,
                                                scalar2=12.0, op0=ALU.max,
                                                op1=ALU.min)
                    gg = small.tile([P, 1], F32, tag="gg")
                    nc.scalar.activation(out=gg, in_=lg, func=ACT.Tanh, scale=0.5)
                    nc.vector.tensor_scalar(out=gg, in0=gg, scalar1=0.5,
                                            scalar2=0.5, op0=ALU.mult, op1=ALU.add)
                    nc.vector.tensor_scalar(out=gg, in0=gg, scalar1=0.05,
                                            scalar2=0.95, op0=ALU.max, op1=ALU.min)

                    # fused = g*a_ctx + (1-g)*rstd_v*w_v
                    t1 = scratch.tile([P, DM], F32, tag="t1")
                    nc.scalar.activation(out=t1, in_=actx, func=ACT.Copy, scale=gg)
                    sgv = small.tile([P, 1], F32, tag="sgv")
                    nc.vector.tensor_scalar(out=sgv, in0=gg,
                                            scalar1=nrstd[:, 0, j:j + 1],
                                            scalar2=rstd_all[:, 0, j:j + 1],
                                            op0=ALU.mult, op1=ALU.add)
                    o_sb = obuf.tile([P, DM], F32)
                    nc.vector.scalar_tensor_tensor(out=o_sb, in0=w_v, scalar=sgv,
                                                   in1=t1, op0=ALU.mult, op1=ALU.add)
                    i_glob = bidx * NB + j
                    nc.gpsimd.dma_start(out=out.ap()[i_glob * P:(i_glob + 1) * P, :],
                                          in_=o_sb)

    nc.compile()
    return nc


def _prepare_in_maps(video_seq, audio_seq, Wv, bv, Wa, ba, theta, W1, b1, W2, b2):
    video_seq = np.asarray(video_seq, np.float32)
    audio_seq = np.asarray(audio_seq, np.float32)
    th = float(np.clip(np.float32(theta), -12.0, 12.0))
    delta = 2.0 + 4.0 / (1.0 + np.exp(-th))
    cmats = _build_cmats(float(delta))

    bv_nz = bool(np.any(np.asarray(bv) != 0))
    ba_nz = bool(np.any(np.asarray(ba) != 0))
    b1_nz = bool(np.any(np.asarray(b1) != 0))
    b2f = float(np.asarray(b2).reshape(-1)[0])

    shared = {
        "wv": np.ascontiguousarray(Wv, np.float32),
        "wa": np.ascontiguousarray(Wa, np.float32),
        "w1": np.ascontiguousarray(
            np.asarray(W1, np.float32).astype(ml_dtypes.bfloat16)),
        "w2h": np.ascontiguousarray(np.asarray(W2, np.float32).reshape(HID)),
        "cm": cmats,
        "ident": np.eye(P, dtype=np.float32).astype(ml_dtypes.bfloat16),
    }
    if bv_nz:
        shared["bvr"] = np.ascontiguousarray(np.asarray(bv, np.float32).reshape(1, DM))
    if ba_nz:
        shared["bar"] = np.ascontiguousarray(np.asarray(ba, np.float32).reshape(1, DM))
    if b1_nz:
        shared["b1r"] = np.ascontiguousarray(np.asarray(b1, np.float32).reshape(1, HID))

    in_maps = []
    for b in range(B):
        m = dict(shared)
        m["vT"] = np.ascontiguousarray(video_seq[b].T)
        m["aT"] = np.ascontiguousarray(audio_seq[b].T)
        in_maps.append(m)
    return in_maps, (bv_nz, ba_nz, b1_nz, b2f)


def kernel(video_seq, audio_seq, Wv, bv, Wa, ba, theta, W1, b1, W2, b2):
    in_maps, key = _prepare_in_maps(video_seq, audio_seq, Wv, bv, Wa, ba,
                                    theta, W1, b1, W2, b2)
    if key not in _nc_cache:
        _nc_cache[key] = _build(*key)
    nc = _nc_cache[key]
    res = run_bass_kernel_spmd(nc, in_maps, list(range(B)))
    return np.stack([res.results[i]["out"] for i in range(B)])



# revision 7
# speedup vs baseline: 1.0949x; 1.0949x over previous
"""Trainium2 Bass/Tile kernel for nn_CAVAModule (cross-attention A/V alignment).

Math notes (exact simplifications of the reference):
  - delta = 2 + 4*sigmoid(clip(theta,-12,12)) is in [2, 6], so the mask
    center min(max(t+delta,0),t) == t for every t: the displacement-aware
    causal mask is a fixed 6-tap causal moving average, independent of theta.
  - The soft temporal shift composed with that moving average is a banded
    Toeplitz operator over time with a <=8-tap band; rows t < 12 deviate
    (clipping at t=0), so shift+mask+einsum collapse into three 128x128
    blocks (C00 / Csub / Cdiag) applied as PE matmuls per 128-token tile.
  - Host pre-centers Wv/Wa columns (Wv_c = Wv - rowmean(Wv)), which makes
    the projection output mean-centered for free: LayerNorm reduces to a
    variance-only rescale, so bn_stats / bn_aggr / the mean-subtract pass
    all disappear.  var = ssq/DM exactly (mean == 0 by construction).
  - All rsqrt-type per-token scalars (rstd_v, rstd_a, 1/||w_v||, 1/||a_ctx||)
    are computed with a Newton-Raphson rsqrt on the Vector engine (bit-magic
    seed + one NR step, max rel err ~1.7e-3): the ACT Sqrt table set never
    loads, so the Scalar engine keeps one table set (Gelu/Tanh/Copy) for the
    whole kernel -- no ACT_TABLE_LOAD stalls.
  - l2_normalize(LN(x)) = l2_normalize(x - mu): the LN scale cancels; the
    final (1-g)*v term is reconstructed as w_v * ((1-g)*rstd_v).
  - bf16 carriers everywhere off the f32r projection matmuls; end-to-end
    rel err ~2.7e-3 (budget 2e-2).

Sharding: data-parallel over batch, one sample per NeuronCore, no cross-core
communication. 16 tiles of 128 tokens, processed in batches of NB=4.
"""

import sys

for _p in ("/opt/trn_rl_repo",):
    if _p not in sys.path:
        sys.path.insert(0, _p)

import ml_dtypes
import numpy as np

import concourse.bacc as bacc
import concourse.bass as bass
import concourse.tile as tile
from concourse import mybir
from concourse.bass_utils import run_bass_kernel_spmd

F32 = mybir.dt.float32
F32R = mybir.dt.float32r
BF16 = mybir.dt.bfloat16
I32 = mybir.dt.int32
ALU = mybir.AluOpType
ACT = mybir.ActivationFunctionType

B, T, VDIM, ADIM, DM = 8, 2048, 1024, 768, 256
HID = 1024
P = 128
NT = T // P          # 16 token tiles
KV = VDIM // P       # 8
KA = ADIM // P       # 6
KX = (3 * DM) // P   # 6
NB = 4               # token tiles per batch (input DMA chunk + scalar batch)
NBAT = NT // NB      # 4
LN_EPS = 1e-5
WIN = 6              # mask window taps (tau in [t-5, t])
MAGIC1 = 0x5F3759E0  # quake rsqrt magic + 1 (for the ~(x>>1) + M+1 form)

_nc_cache: dict = {}


def _build_cmats(delta: float) -> np.ndarray:
    """Three [tau, t] blocks of the combined shift+mask operator."""
    dl = min(max(delta, 0.0), float(T - 1))
    n = int(np.floor(dl))
    alpha = dl - n

    def row_w(t):
        w = np.zeros(2 * P, np.float64)
        m = min(t + 1, WIN)
        for s in range(max(0, t - (WIN - 1)), t + 1):
            i0 = min(max(s - n, 0), T - 1)
            i1 = min(i0 + 1, T - 1)
            w[i0] += (1.0 - alpha) / m
            w[i1] += alpha / m
        return w

    c00 = np.zeros((P, P), np.float64)
    csub = np.zeros((P, P), np.float64)
    cdiag = np.zeros((P, P), np.float64)
    for t in range(P):
        w = row_w(t)
        c00[:, t] = w[:P]
        w = row_w(P + t)
        csub[:, t] = w[:P]
        cdiag[:, t] = w[P:2 * P]
    return np.ascontiguousarray(np.stack([c00, csub, cdiag]).astype(np.float32))


def _build(bv_nz: bool, ba_nz: bool, b1_nz: bool, b2f: float):
    nc = bacc.Bacc("TRN2", target_bir_lowering=False, debug=False, num_devices=8)

    vT = nc.dram_tensor("vT", [VDIM, T], F32R, kind="ExternalInput")
    aT = nc.dram_tensor("aT", [ADIM, T], F32R, kind="ExternalInput")
    wv = nc.dram_tensor("wv", [VDIM, DM], F32R, kind="ExternalInput")
    wa = nc.dram_tensor("wa", [ADIM, DM], F32R, kind="ExternalInput")
    w1 = nc.dram_tensor("w1", [3 * DM, HID], BF16, kind="ExternalInput")
    w2h = nc.dram_tensor("w2h", [HID], BF16, kind="ExternalInput")
    cm = nc.dram_tensor("cm", [3, P, P], BF16, kind="ExternalInput")
    ident = nc.dram_tensor("ident", [P, P], BF16, kind="ExternalInput")
    if bv_nz:
        bvr = nc.dram_tensor("bvr", [1, DM], F32R, kind="ExternalInput")
    if ba_nz:
        bar = nc.dram_tensor("bar", [1, DM], F32R, kind="ExternalInput")
    if b1_nz:
        b1r = nc.dram_tensor("b1r", [1, HID], F32R, kind="ExternalInput")
    out = nc.dram_tensor("out", [T, DM], F32, kind="ExternalOutput")

    def bcast(handle_ap, n):
        return bass.AP(
            tensor=handle_ap.tensor, offset=handle_ap.offset, ap=[[0, P], [1, n]]
        )

    from contextlib import ExitStack

    with tile.TileContext(nc) as tc:
        with ExitStack() as stack:
            pool = lambda *a, **k: stack.enter_context(tc.tile_pool(*a, **k))
            singles = pool(name="singles", bufs=1)
            vchunk = pool(name="vchunk", bufs=2)
            achunk = pool(name="achunk", bufs=2)
            wv_pool = pool(name="wvbf", bufs=NB + 2)   # centered video, bf16
            wa_pool = pool(name="wabf", bufs=NB + 1)   # centered audio, bf16
            asb_pool = pool(name="asb", bufs=3)        # LN'd audio, bf16
            acx_pool = pool(name="acxp", bufs=NB + 2)  # a_ctx, bf16
            vn_pool = pool(name="vnp", bufs=3)
            an_pool = pool(name="anp", bufs=3)
            xt_pool = pool(name="xtp", bufs=2)
            hbuf = pool(name="hb", bufs=2)
            lsc_pool = pool(name="lscp", bufs=2)
            t2_pool = pool(name="t2p", bufs=3)
            obuf = pool(name="ob", bufs=3)
            scratch = pool(name="scr", bufs=2)         # ttr throwaway
            batch_small = pool(name="bsm", bufs=2)
            psum_mm = pool(name="psum_mm", bufs=3, space="PSUM")   # pv/pa
            psum_ct = pool(name="psum_ct", bufs=3, space="PSUM")   # pc/pt
            psum_h = pool(name="psum_h", bufs=2, space="PSUM")
            # ---- persistent weights/constants ----
            wv_sb = singles.tile([P, KV, DM], F32R)
            nc.gpsimd.dma_start(out=wv_sb, in_=wv.ap().rearrange("(ko p) n -> p ko n", p=P))
            wa_sb = singles.tile([P, KA, DM], F32R)
            nc.gpsimd.dma_start(out=wa_sb, in_=wa.ap().rearrange("(ko p) n -> p ko n", p=P))
            w1_sb = singles.tile([P, KX, HID], BF16)
            nc.gpsimd.dma_start(out=w1_sb, in_=w1.ap().rearrange("(ko p) n -> p ko n", p=P))
            w2_sb = singles.tile([P, HID], BF16)
            nc.gpsimd.dma_start(out=w2_sb, in_=bcast(w2h.ap(), HID))
            cm_sb = singles.tile([P, 3, P], BF16)
            nc.gpsimd.dma_start(out=cm_sb, in_=cm.ap().rearrange("c p t -> p c t"))
            id_sb = singles.tile([P, P], BF16)
            nc.gpsimd.dma_start(out=id_sb, in_=ident.ap())
            if bv_nz or ba_nz or b1_nz:
                ones_sb = singles.tile([1, P], F32R)
                nc.vector.memset(ones_sb, 1.0)
            if bv_nz:
                bv_sb = singles.tile([1, DM], F32R)
                nc.sync.dma_start(out=bv_sb, in_=bvr.ap())
            if ba_nz:
                ba_sb = singles.tile([1, DM], F32R)
                nc.sync.dma_start(out=ba_sb, in_=bar.ap())
            if b1_nz:
                b1_sb = singles.tile([1, HID], F32R)
                nc.sync.dma_start(out=b1_sb, in_=b1r.ap())

            vT_r = vT.ap().rearrange("(ko p) t -> p ko t", p=P)
            aT_r = aT.ap().rearrange("(ko p) t -> p ko t", p=P)

            a_prev = None
            for bidx in range(NBAT):
                tsl_g = slice(bidx * NB * P, (bidx + 1) * NB * P)
                vt_sb = vchunk.tile([P, KV, NB * P], F32R)
                nc.sync.dma_start(out=vt_sb, in_=vT_r[:, :, tsl_g])
                at_sb = achunk.tile([P, KA, NB * P], F32R)
                nc.sync.dma_start(out=at_sb, in_=aT_r[:, :, tsl_g])

                # batch scalar tiles
                # rs layout: [0:NB]=varpe_v, [NB:2NB]=varpe_a, [2NB:3NB]=ssq_v
                rs = batch_small.tile([P, 3 * NB], F32, tag="rs")
                rr = batch_small.tile([P, 3 * NB], F32, tag="rr")   # rsqrt(rs)
                nrt = batch_small.tile([P, 3 * NB], F32, tag="nrt") # NR temps
                cs = batch_small.tile([P, NB], F32, tag="cs")       # ssq_actx
                rna = batch_small.tile([P, NB], F32, tag="rna")
                nrt2 = batch_small.tile([P, NB], F32, tag="nrt2")
                lgb = batch_small.tile([P, NB], F32, tag="lgb")     # gate logits
                ggc = batch_small.tile([P, NB], F32, tag="ggc")     # clipped gate
                m2c = batch_small.tile([P, NB], F32, tag="m2c")     # (1-g)*rstd_v

                wv_t, wa_t = [], []
                # ---- phase A: projections + evacuate + ssq ----
                for j in range(NB):
                    tsl = slice(j * P, (j + 1) * P)
                    pv = psum_mm.tile([P, DM], F32, tag="mm")
                    for k in range(KV):
                        nc.tensor.matmul(pv, lhsT=vt_sb[:, k, tsl],
                                         rhs=wv_sb[:, k, :],
                                         start=(k == 0),
                                         stop=(k == KV - 1 and not bv_nz))
                    if bv_nz:
                        nc.tensor.matmul(pv, lhsT=ones_sb, rhs=bv_sb,
                                         start=False, stop=True)
                    w_v = wv_pool.tile([P, DM], BF16)
                    nc.scalar.copy(out=w_v, in_=pv)
                    sq = scratch.tile([P, DM], F32, tag="sq")
                    nc.vector.scalar_tensor_tensor(
                        out=sq, in0=w_v, scalar=1.0, in1=w_v,
                        op0=ALU.mult, op1=ALU.mult,
                        accum_out=rs[:, 2 * NB + j:2 * NB + j + 1])
                    wv_t.append(w_v)

                    pa = psum_mm.tile([P, DM], F32, tag="mm")
                    for k in range(KA):
                        nc.tensor.matmul(pa, lhsT=at_sb[:, k, tsl],
                                         rhs=wa_sb[:, k, :],
                                         start=(k == 0),
                                         stop=(k == KA - 1 and not ba_nz))
                    if ba_nz:
                        nc.tensor.matmul(pa, lhsT=ones_sb, rhs=ba_sb,
                                         start=False, stop=True)
                    w_a = wa_pool.tile([P, DM], BF16)
                    nc.scalar.copy(out=w_a, in_=pa)
                    sq2 = scratch.tile([P, DM], F32, tag="sq")
                    nc.vector.scalar_tensor_tensor(
                        out=sq2, in0=w_a, scalar=1.0, in1=w_a,
                        op0=ALU.mult, op1=ALU.mult,
                        accum_out=rs[:, NB + j:NB + j + 1])
                    wa_t.append(w_a)

                # varpe_v = ssq_v/DM + eps; varpe_a = ssq_a/DM + eps
                nc.vector.tensor_scalar(out=rs[:, 0:NB], in0=rs[:, 2 * NB:3 * NB],
                                        scalar1=1.0 / DM, scalar2=LN_EPS,
                                        op0=ALU.mult, op1=ALU.add)
                nc.vector.tensor_scalar(out=rs[:, NB:2 * NB], in0=rs[:, NB:2 * NB],
                                        scalar1=1.0 / DM, scalar2=LN_EPS,
                                        op0=ALU.mult, op1=ALU.add)
                # ---- NR rsqrt round 1 on [P, 3NB]: rstd_v | rstd_a | rn_v ----
                rs_i = rs[:].bitcast(I32)
                rr_i = rr[:].bitcast(I32)
                nc.vector.tensor_scalar(out=rr_i, in0=rs_i, scalar1=1,
                                        scalar2=-1, op0=ALU.logical_shift_right,
                                        op1=ALU.bitwise_xor)
                nc.vector.tensor_scalar(out=rr_i, in0=rr_i, scalar1=MAGIC1,
                                        scalar2=None, op0=ALU.add)
                nc.vector.tensor_tensor(out=nrt, in0=rr, in1=rr, op=ALU.mult)
                nc.vector.scalar_tensor_tensor(out=nrt, in0=rs, scalar=-0.5,
                                               in1=nrt, op0=ALU.mult, op1=ALU.mult)
                nc.vector.tensor_scalar(out=nrt, in0=nrt, scalar1=1.5,
                                        scalar2=None, op0=ALU.add)
                nc.vector.tensor_tensor(out=rr, in0=rr, in1=nrt, op=ALU.mult)

                # ---- phase B: audio LN scale, context matmul, ctx ssq ----
                acx_t = []
                for j in range(NB):
                    a_sb = asb_pool.tile([P, DM], BF16)
                    nc.vector.tensor_scalar(out=a_sb, in0=wa_t[j],
                                            scalar1=rr[:, NB + j:NB + j + 1],
                                            scalar2=None, op0=ALU.mult)
                    pc = psum_ct.tile([P, DM], F32, tag="ct")
                    if bidx == 0 and j == 0:
                        nc.tensor.matmul(pc, lhsT=cm_sb[:, 0, :], rhs=a_sb,
                                         start=True, stop=True)
                    else:
                        nc.tensor.matmul(pc, lhsT=cm_sb[:, 1, :], rhs=a_prev,
                                         start=True, stop=False)
                        nc.tensor.matmul(pc, lhsT=cm_sb[:, 2, :], rhs=a_sb,
                                         start=False, stop=True)
                    a_prev = a_sb
                    acx = acx_pool.tile([P, DM], BF16)
                    nc.scalar.copy(out=acx, in_=pc)
                    sq3 = scratch.tile([P, DM], F32, tag="sq")
                    nc.vector.scalar_tensor_tensor(
                        out=sq3, in0=acx, scalar=1.0, in1=acx,
                        op0=ALU.mult, op1=ALU.mult,
                        accum_out=cs[:, j:j + 1])
                    acx_t.append(acx)

                # ---- NR rsqrt round 2 on [P, NB]: rn_a ----
                cs_i = cs[:].bitcast(I32)
                rna_i = rna[:].bitcast(I32)
                nc.vector.tensor_scalar(out=rna_i, in0=cs_i, scalar1=1,
                                        scalar2=-1, op0=ALU.logical_shift_right,
                                        op1=ALU.bitwise_xor)
                nc.vector.tensor_scalar(out=rna_i, in0=rna_i, scalar1=MAGIC1,
                                        scalar2=None, op0=ALU.add)
                nc.vector.tensor_tensor(out=nrt2, in0=rna, in1=rna, op=ALU.mult)
                nc.vector.scalar_tensor_tensor(out=nrt2, in0=cs, scalar=-0.5,
                                               in1=nrt2, op0=ALU.mult, op1=ALU.mult)
                nc.vector.tensor_scalar(out=nrt2, in0=nrt2, scalar1=1.5,
                                        scalar2=None, op0=ALU.add)
                nc.vector.tensor_tensor(out=rna, in0=rna, in1=nrt2, op=ALU.mult)

                # ---- phase C: gate features, transposes, MLP, dot ----
                for j in range(NB):
                    w_v = wv_t[j]
                    acx = acx_t[j]
                    vn = vn_pool.tile([P, DM], BF16)
                    nc.vector.tensor_scalar(out=vn, in0=w_v,
                                            scalar1=rr[:, 2 * NB + j:2 * NB + j + 1],
                                            scalar2=None, op0=ALU.mult)
                    an = an_pool.tile([P, DM], BF16)
                    nc.vector.tensor_scalar(out=an, in0=acx,
                                            scalar1=rna[:, j:j + 1],
                                            scalar2=None, op0=ALU.mult)

                    pt = psum_ct.tile([P, 4, P], BF16, tag="ct")
                    for k in range(2):
                        nc.tensor.transpose(pt[:, k, :], an[:, k * P:(k + 1) * P], id_sb)
                    for k in range(2):
                        nc.tensor.transpose(pt[:, 2 + k, :], vn[:, k * P:(k + 1) * P], id_sb)
                    xt = xt_pool.tile([P, KX, P], BF16)
                    nc.vector.tensor_copy(out=xt[:, 0:4, :], in_=pt)
                    nc.gpsimd.tensor_mul(out=xt[:, 4:6, :], in0=xt[:, 0:2, :],
                                         in1=xt[:, 2:4, :])

                    hh = hbuf.tile([P, HID], BF16, tag="hh")
                    for nh in range(2):
                        ph = psum_h.tile([P, 512], F32, tag="ph")
                        for k in range(KX):
                            nc.tensor.matmul(
                                ph, lhsT=xt[:, k, :],
                                rhs=w1_sb[:, k, nh * 512:(nh + 1) * 512],
                                start=(k == 0), stop=(k == KX - 1 and not b1_nz))
                        if b1_nz:
                            nc.tensor.matmul(ph, lhsT=ones_sb,
                                             rhs=b1_sb[:, nh * 512:(nh + 1) * 512],
                                             start=False, stop=True)
                        nc.scalar.activation(out=hh[:, nh * 512:(nh + 1) * 512],
                                             in_=ph, func=ACT.Gelu)
                    lsc = lsc_pool.tile([P, HID], BF16)
                    nc.vector.scalar_tensor_tensor(out=lsc, in0=hh, scalar=0.0,
                                                   in1=w2_sb, op0=ALU.bypass,
                                                   op1=ALU.mult,
                                                   accum_out=lgb[:, j:j + 1])

                # ---- batched gate tail ----
                if b2f != 0.0:
                    nc.vector.tensor_scalar(out=lgb, in0=lgb, scalar1=float(b2f),
                                            scalar2=-12.0, op0=ALU.add, op1=ALU.max)
                    nc.vector.tensor_scalar(out=lgb, in0=lgb, scalar1=12.0,
                                            scalar2=None, op0=ALU.min)
                else:
                    nc.vector.tensor_scalar(out=lgb, in0=lgb, scalar1=-12.0,
                                            scalar2=12.0, op0=ALU.max, op1=ALU.min)
                nc.scalar.activation(out=ggc, in_=lgb, func=ACT.Tanh, scale=0.5)
                nc.vector.tensor_scalar(out=ggc, in0=ggc, scalar1=0.5,
                                        scalar2=0.5, op0=ALU.mult, op1=ALU.add)
                nc.vector.tensor_scalar(out=ggc, in0=ggc, scalar1=0.05,
                                        scalar2=0.95, op0=ALU.max, op1=ALU.min)
                # m2 = (1 - g) * rstd_v
                nc.vector.tensor_scalar(out=m2c, in0=ggc, scalar1=-1.0,
                                        scalar2=1.0, op0=ALU.mult, op1=ALU.add)
                nc.vector.tensor_tensor(out=m2c, in0=m2c, in1=rr[:, 0:NB],
                                        op=ALU.mult)

                # ---- fuse + output ----
                for j in range(NB):
                    t2 = t2_pool.tile([P, DM], BF16)
                    nc.vector.tensor_scalar(out=t2, in0=wv_t[j],
                                            scalar1=m2c[:, j:j + 1],
                                            scalar2=None, op0=ALU.mult)
                    o_sb = obuf.tile([P, DM], F32)
                    nc.vector.scalar_tensor_tensor(out=o_sb, in0=acx_t[j],
                                                   scalar=ggc[:, j:j + 1],
                                                   in1=t2, op0=ALU.mult,
                                                   op1=ALU.add)
                    i_glob = bidx * NB + j
                    nc.sync.dma_start(out=out.ap()[i_glob * P:(i_glob + 1) * P, :],
                                      in_=o_sb)

    nc.compile()
    return nc


def _prepare_in_maps(video_seq, audio_seq, Wv, bv, Wa, ba, theta, W1, b1, W2, b2):
    video_seq = np.asarray(video_seq, np.float32)
    audio_seq = np.asarray(audio_seq, np.float32)
    th = float(np.clip(np.float32(theta), -12.0, 12.0))
    delta = 2.0 + 4.0 / (1.0 + np.exp(-th))
    cmats = _build_cmats(float(delta))

    Wv = np.asarray(Wv, np.float32)
    Wa = np.asarray(Wa, np.float32)
    bv = np.asarray(bv, np.float32)
    ba = np.asarray(ba, np.float32)
    # pre-center output features: mean over DM commutes through the matmul
    Wv_c = Wv - Wv.mean(axis=1, keepdims=True)
    Wa_c = Wa - Wa.mean(axis=1, keepdims=True)
    bv_c = bv - bv.mean()
    ba_c = ba - ba.mean()

    bv_nz = bool(np.any(bv_c != 0))
    ba_nz = bool(np.any(ba_c != 0))
    b1_nz = bool(np.any(np.asarray(b1) != 0))
    b2f = float(np.asarray(b2).reshape(-1)[0])

    shared = {
        "wv": np.ascontiguousarray(Wv_c),
        "wa": np.ascontiguousarray(Wa_c),
        "w1": np.ascontiguousarray(
            np.asarray(W1, np.float32).astype(ml_dtypes.bfloat16)),
        "w2h": np.ascontiguousarray(
            np.asarray(W2, np.float32).reshape(HID).astype(ml_dtypes.bfloat16)),
        "cm": np.ascontiguousarray(cmats.astype(ml_dtypes.bfloat16)),
        "ident": np.eye(P, dtype=np.float32).astype(ml_dtypes.bfloat16),
    }
    if bv_nz:
        shared["bvr"] = np.ascontiguousarray(bv_c.reshape(1, DM))
    if ba_nz:
        shared["bar"] = np.ascontiguousarray(ba_c.reshape(1, DM))
    if b1_nz:
        shared["b1r"] = np.ascontiguousarray(np.asarray(b1, np.float32).reshape(1, HID))

    in_maps = []
    for b in range(B):
        m = dict(shared)
        m["vT"] = np.ascontiguousarray(video_seq[b].T)
        m["aT"] = np.ascontiguousarray(audio_seq[b].T)
        in_maps.append(m)
    return in_maps, (bv_nz, ba_nz, b1_nz, b2f)


def kernel(video_seq, audio_seq, Wv, bv, Wa, ba, theta, W1, b1, W2, b2):
    in_maps, key = _prepare_in_maps(video_seq, audio_seq, Wv, bv, Wa, ba,
                                    theta, W1, b1, W2, b2)
    if key not in _nc_cache:
        _nc_cache[key] = _build(*key)
    nc = _nc_cache[key]
    res = run_bass_kernel_spmd(nc, in_maps, list(range(B)))
    return np.stack([res.results[i]["out"] for i in range(B)])


# revision 11
# speedup vs baseline: 1.1576x; 1.0573x over previous
"""Trainium2 Bass/Tile kernel for nn_CAVAModule (cross-attention A/V alignment).

Math notes (exact simplifications of the reference):
  - delta = 2 + 4*sigmoid(clip(theta,-12,12)) is in [2, 6], so the mask
    center min(max(t+delta,0),t) == t for every t: the displacement-aware
    causal mask is a fixed 6-tap causal moving average, independent of theta.
  - The soft temporal shift composed with that moving average is a banded
    Toeplitz operator over time with a <=8-tap band; rows t < 12 deviate
    (clipping at t=0), so shift+mask+einsum collapse into three 128x128
    blocks (C00 / Csub / Cdiag) applied as PE matmuls per 128-token tile.
  - Host pre-centers Wv/Wa columns (Wv_c = Wv - rowmean(Wv)), which makes
    the projection output mean-centered for free: LayerNorm reduces to a
    variance-only rescale, so bn_stats / bn_aggr / the mean-subtract pass
    all disappear.  var = ssq/DM exactly (mean == 0 by construction).
  - All rsqrt-type per-token scalars (rstd_v, rstd_a, 1/||w_v||, 1/||a_ctx||)
    are computed with a Newton-Raphson rsqrt on the Vector engine (bit-magic
    seed + one NR step, max rel err ~1.7e-3): the ACT Sqrt table set never
    loads, so the Scalar engine keeps one table set (Gelu/Tanh/Copy) for the
    whole kernel -- no ACT_TABLE_LOAD stalls.
  - l2_normalize(LN(x)) = l2_normalize(x - mu): the LN scale cancels; the
    final (1-g)*v term is reconstructed as w_v * ((1-g)*rstd_v).
  - bf16 carriers everywhere off the f32r projection matmuls; end-to-end
    rel err ~2.7e-3 (budget 2e-2).

Sharding: data-parallel over batch, one sample per NeuronCore, no cross-core
communication. 16 tiles of 128 tokens, processed in batches of NB=4.
"""

import sys

for _p in ("/opt/trn_rl_repo",):
    if _p not in sys.path:
        sys.path.insert(0, _p)

import ml_dtypes
import numpy as np

import concourse.bacc as bacc
import concourse.bass as bass
import concourse.tile as tile
from concourse import mybir
from concourse.bass_utils import run_bass_kernel_spmd

F32 = mybir.dt.float32
F32R = mybir.dt.float32r
BF16 = mybir.dt.bfloat16
I32 = mybir.dt.int32
ALU = mybir.AluOpType
ACT = mybir.ActivationFunctionType

B, T, VDIM, ADIM, DM = 8, 2048, 1024, 768, 256
HID = 1024
P = 128
NT = T // P          # 16 token tiles
KV = VDIM // P       # 8
KA = ADIM // P       # 6
KX = (3 * DM) // P   # 6
NB = 8               # token tiles per batch (scalar batch)
NBAT = NT // NB      # 2
LN_EPS = 1e-5
WIN = 6              # mask window taps (tau in [t-5, t])
MAGIC1 = 0x5F3759E0  # quake rsqrt magic + 1 (for the ~(x>>1) + M+1 form)

_nc_cache: dict = {}


def _build_cmats(delta: float) -> np.ndarray:
    """Three [tau, t] blocks of the combined shift+mask operator."""
    dl = min(max(delta, 0.0), float(T - 1))
    n = int(np.floor(dl))
    alpha = dl - n

    def row_w(t):
        w = np.zeros(2 * P, np.float64)
        m = min(t + 1, WIN)
        for s in range(max(0, t - (WIN - 1)), t + 1):
            i0 = min(max(s - n, 0), T - 1)
            i1 = min(i0 + 1, T - 1)
            w[i0] += (1.0 - alpha) / m
            w[i1] += alpha / m
        return w

    c00 = np.zeros((P, P), np.float64)
    csub = np.zeros((P, P), np.float64)
    cdiag = np.zeros((P, P), np.float64)
    for t in range(P):
        w = row_w(t)
        c00[:, t] = w[:P]
        w = row_w(P + t)
        csub[:, t] = w[:P]
        cdiag[:, t] = w[P:2 * P]
    return np.ascontiguousarray(np.stack([c00, csub, cdiag]).astype(np.float32))


def _build(bv_nz: bool, ba_nz: bool, b1_nz: bool, b2f: float):
    nc = bacc.Bacc("TRN2", target_bir_lowering=False, debug=False, num_devices=8)

    vT = nc.dram_tensor("vT", [VDIM, T], F32R, kind="ExternalInput")
    aT = nc.dram_tensor("aT", [ADIM, T], F32R, kind="ExternalInput")
    wv = nc.dram_tensor("wv", [VDIM, DM], F32R, kind="ExternalInput")
    wa = nc.dram_tensor("wa", [ADIM, DM], F32R, kind="ExternalInput")
    w1 = nc.dram_tensor("w1", [3 * DM, HID], BF16, kind="ExternalInput")
    w2h = nc.dram_tensor("w2h", [HID], BF16, kind="ExternalInput")
    cm = nc.dram_tensor("cm", [3, P, P], BF16, kind="ExternalInput")
    ident = nc.dram_tensor("ident", [P, P], BF16, kind="ExternalInput")
    if bv_nz:
        bvr = nc.dram_tensor("bvr", [1, DM], F32R, kind="ExternalInput")
    if ba_nz:
        bar = nc.dram_tensor("bar", [1, DM], F32R, kind="ExternalInput")
    if b1_nz:
        b1r = nc.dram_tensor("b1r", [1, HID], F32R, kind="ExternalInput")
    out = nc.dram_tensor("out", [T, DM], F32, kind="ExternalOutput")

    def bcast(handle_ap, n):
        return bass.AP(
            tensor=handle_ap.tensor, offset=handle_ap.offset, ap=[[0, P], [1, n]]
        )

    from contextlib import ExitStack

    with tile.TileContext(nc) as tc:
        with ExitStack() as stack:
            pool = lambda *a, **k: stack.enter_context(tc.tile_pool(*a, **k))
            singles = pool(name="singles", bufs=1)
            vchunk = pool(name="vchunk", bufs=6)       # per-pair vt slices
            achunk = pool(name="achunk", bufs=6)
            wv_pool = pool(name="wvbf", bufs=NB + 2)   # centered video, bf16
            wa_pool = pool(name="wabf", bufs=NB + 2)   # centered audio, bf16
            asb_pool = pool(name="asb", bufs=4)        # LN'd audio, bf16
            acx_pool = pool(name="acxp", bufs=NB + 2)  # a_ctx, bf16
            vn_pool = pool(name="vnp", bufs=4)
            an_pool = pool(name="anp", bufs=4)
            xt_pool = pool(name="xtp", bufs=3)
            hbuf = pool(name="hb", bufs=3)
            lsc_pool = pool(name="lscp", bufs=2)
            t2_pool = pool(name="t2p", bufs=3)
            obuf = pool(name="ob", bufs=4)
            scratch = pool(name="scr", bufs=2)         # ttr throwaway
            batch_small = pool(name="bsm", bufs=2)
            psum_mm = pool(name="psum_mm", bufs=3, space="PSUM")   # pv/pa
            psum_ct = pool(name="psum_ct", bufs=3, space="PSUM")   # pc/pt
            psum_h = pool(name="psum_h", bufs=2, space="PSUM")
            vT_r = vT.ap().rearrange("(ko p) t -> p ko t", p=P)
            aT_r = aT.ap().rearrange("(ko p) t -> p ko t", p=P)

            # per-pair (2-tile) input chunks; startup interleaves the first
            # pairs with the weight loads so the PE starts ASAP
            pair_tiles: dict = {}

            def pair_dma(bidx, c):
                t0 = (bidx * NB + 2 * c) * P
                vp = vchunk.tile([P, KV, 2 * P], F32R)
                nc.sync.dma_start(out=vp, in_=vT_r[:, :, t0:t0 + 2 * P])
                ap_ = achunk.tile([P, KA, 2 * P], F32R)
                nc.sync.dma_start(out=ap_, in_=aT_r[:, :, t0:t0 + 2 * P])
                pair_tiles[(bidx, c)] = (vp, ap_)

            # ---- startup order: first data pair, core weights, rest ----
            wv_sb = singles.tile([P, KV, DM], F32R)
            nc.gpsimd.dma_start(out=wv_sb, in_=wv.ap().rearrange("(ko p) n -> p ko n", p=P))
            pair_dma(0, 0)
            wa_sb = singles.tile([P, KA, DM], F32R)
            nc.gpsimd.dma_start(out=wa_sb, in_=wa.ap().rearrange("(ko p) n -> p ko n", p=P))
            pair_dma(0, 1)
            cm_sb = singles.tile([P, 3, P], BF16)
            nc.gpsimd.dma_start(out=cm_sb, in_=cm.ap().rearrange("c p t -> p c t"))
            id_sb = singles.tile([P, P], BF16)
            nc.gpsimd.dma_start(out=id_sb, in_=ident.ap())
            pair_dma(0, 2)
            pair_dma(0, 3)
            w1_sb = singles.tile([P, KX, HID], BF16)
            nc.gpsimd.dma_start(out=w1_sb, in_=w1.ap().rearrange("(ko p) n -> p ko n", p=P))
            w2_sb = singles.tile([P, HID], BF16)
            nc.gpsimd.dma_start(out=w2_sb, in_=bcast(w2h.ap(), HID))
            if bv_nz or ba_nz or b1_nz:
                ones_sb = singles.tile([1, P], F32R)
                nc.vector.memset(ones_sb, 1.0)
            if bv_nz:
                bv_sb = singles.tile([1, DM], F32R)
                nc.sync.dma_start(out=bv_sb, in_=bvr.ap())
            if ba_nz:
                ba_sb = singles.tile([1, DM], F32R)
                nc.sync.dma_start(out=ba_sb, in_=bar.ap())
            if b1_nz:
                b1_sb = singles.tile([1, HID], F32R)
                nc.sync.dma_start(out=b1_sb, in_=b1r.ap())

            a_prev = None
            for bidx in range(NBAT):
                for c in range(NB // 2):
                    if (bidx, c) not in pair_tiles:
                        pair_dma(bidx, c)

                # batch scalar tiles
                # rs layout: [0:NB]=varpe_v, [NB:2NB]=varpe_a, [2NB:3NB]=ssq_v
                rs = batch_small.tile([P, 3 * NB], F32, tag="rs")
                rr = batch_small.tile([P, 3 * NB], F32, tag="rr")   # rsqrt(rs)
                nrt = batch_small.tile([P, 3 * NB], F32, tag="nrt") # NR temps
                cs = batch_small.tile([P, NB], F32, tag="cs")       # ssq_actx
                rna = batch_small.tile([P, NB], F32, tag="rna")
                nrt2 = batch_small.tile([P, NB], F32, tag="nrt2")
                lgb = batch_small.tile([P, NB], F32, tag="lgb")     # gate logits
                ggc = batch_small.tile([P, NB], F32, tag="ggc")     # clipped gate
                m2c = batch_small.tile([P, NB], F32, tag="m2c")     # (1-g)*rstd_v

                wv_t, wa_t = [], []
                # ---- phase A: projections + evacuate + ssq ----
                for j in range(NB):
                    vt_sb, at_sb = pair_tiles[(bidx, j // 2)]
                    tsl = slice((j % 2) * P, (j % 2 + 1) * P)
                    pv = psum_mm.tile([P, DM], F32, tag="mm")
                    for k in range(KV):
                        nc.tensor.matmul(pv, lhsT=vt_sb[:, k, tsl],
                                         rhs=wv_sb[:, k, :],
                                         start=(k == 0),
                                         stop=(k == KV - 1 and not bv_nz))
                    if bv_nz:
                        nc.tensor.matmul(pv, lhsT=ones_sb, rhs=bv_sb,
                                         start=False, stop=True)
                    w_v = wv_pool.tile([P, DM], BF16)
                    nc.scalar.copy(out=w_v, in_=pv)
                    sq = scratch.tile([P, DM], F32, tag="sq")
                    nc.vector.scalar_tensor_tensor(
                        out=sq, in0=w_v, scalar=1.0, in1=w_v,
                        op0=ALU.mult, op1=ALU.mult,
                        accum_out=rs[:, 2 * NB + j:2 * NB + j + 1])
                    wv_t.append(w_v)

                    pa = psum_mm.tile([P, DM], F32, tag="mm")
                    for k in range(KA):
                        nc.tensor.matmul(pa, lhsT=at_sb[:, k, tsl],
                                         rhs=wa_sb[:, k, :],
                                         start=(k == 0),
                                         stop=(k == KA - 1 and not ba_nz))
                    if ba_nz:
                        nc.tensor.matmul(pa, lhsT=ones_sb, rhs=ba_sb,
                                         start=False, stop=True)
                    w_a = wa_pool.tile([P, DM], BF16)
                    nc.scalar.copy(out=w_a, in_=pa)
                    sq2 = scratch.tile([P, DM], F32, tag="sq")
                    nc.vector.scalar_tensor_tensor(
                        out=sq2, in0=w_a, scalar=1.0, in1=w_a,
                        op0=ALU.mult, op1=ALU.mult,
                        accum_out=rs[:, NB + j:NB + j + 1])
                    wa_t.append(w_a)

                # varpe_v = ssq_v/DM + eps; varpe_a = ssq_a/DM + eps
                nc.vector.tensor_scalar(out=rs[:, 0:NB], in0=rs[:, 2 * NB:3 * NB],
                                        scalar1=1.0 / DM, scalar2=LN_EPS,
                                        op0=ALU.mult, op1=ALU.add)
                nc.vector.tensor_scalar(out=rs[:, NB:2 * NB], in0=rs[:, NB:2 * NB],
                                        scalar1=1.0 / DM, scalar2=LN_EPS,
                                        op0=ALU.mult, op1=ALU.add)
                # ---- NR rsqrt round 1 on [P, 3NB]: rstd_v | rstd_a | rn_v ----
                rs_i = rs[:].bitcast(I32)
                rr_i = rr[:].bitcast(I32)
                nc.vector.tensor_scalar(out=rr_i, in0=rs_i, scalar1=1,
                                        scalar2=-1, op0=ALU.logical_shift_right,
                                        op1=ALU.bitwise_xor)
                nc.vector.tensor_scalar(out=rr_i, in0=rr_i, scalar1=MAGIC1,
                                        scalar2=None, op0=ALU.add)
                nc.vector.tensor_tensor(out=nrt, in0=rr, in1=rr, op=ALU.mult)
                nc.vector.scalar_tensor_tensor(out=nrt, in0=rs, scalar=-0.5,
                                               in1=nrt, op0=ALU.mult, op1=ALU.mult)
                nc.vector.tensor_scalar(out=nrt, in0=nrt, scalar1=1.5,
                                        scalar2=None, op0=ALU.add)
                nc.vector.tensor_tensor(out=rr, in0=rr, in1=nrt, op=ALU.mult)

                # ---- phase B: audio LN scale, context matmul, ctx ssq ----
                acx_t = []
                for j in range(NB):
                    a_sb = asb_pool.tile([P, DM], BF16)
                    nc.vector.tensor_scalar(out=a_sb, in0=wa_t[j],
                                            scalar1=rr[:, NB + j:NB + j + 1],
                                            scalar2=None, op0=ALU.mult)
                    pc = psum_ct.tile([P, DM], F32, tag="ct")
                    if bidx == 0 and j == 0:
                        nc.tensor.matmul(pc, lhsT=cm_sb[:, 0, :], rhs=a_sb,
                                         start=True, stop=True)
                    else:
                        nc.tensor.matmul(pc, lhsT=cm_sb[:, 1, :], rhs=a_prev,
                                         start=True, stop=False)
                        nc.tensor.matmul(pc, lhsT=cm_sb[:, 2, :], rhs=a_sb,
                                         start=False, stop=True)
                    a_prev = a_sb
                    acx = acx_pool.tile([P, DM], BF16)
                    nc.scalar.copy(out=acx, in_=pc)
                    sq3 = scratch.tile([P, DM], F32, tag="sq")
                    nc.vector.scalar_tensor_tensor(
                        out=sq3, in0=acx, scalar=1.0, in1=acx,
                        op0=ALU.mult, op1=ALU.mult,
                        accum_out=cs[:, j:j + 1])
                    acx_t.append(acx)

                # ---- NR rsqrt round 2 on [P, NB]: rn_a ----
                cs_i = cs[:].bitcast(I32)
                rna_i = rna[:].bitcast(I32)
                nc.vector.tensor_scalar(out=rna_i, in0=cs_i, scalar1=1,
                                        scalar2=-1, op0=ALU.logical_shift_right,
                                        op1=ALU.bitwise_xor)
                nc.vector.tensor_scalar(out=rna_i, in0=rna_i, scalar1=MAGIC1,
                                        scalar2=None, op0=ALU.add)
                nc.vector.tensor_tensor(out=nrt2, in0=rna, in1=rna, op=ALU.mult)
                nc.vector.scalar_tensor_tensor(out=nrt2, in0=cs, scalar=-0.5,
                                               in1=nrt2, op0=ALU.mult, op1=ALU.mult)
                nc.vector.tensor_scalar(out=nrt2, in0=nrt2, scalar1=1.5,
                                        scalar2=None, op0=ALU.add)
                nc.vector.tensor_tensor(out=rna, in0=rna, in1=nrt2, op=ALU.mult)

                # ---- phase C: gate features, transposes, MLP, dot ----
                for j in range(NB):
                    w_v = wv_t[j]
                    acx = acx_t[j]
                    vn = vn_pool.tile([P, DM], BF16)
                    nc.vector.tensor_scalar(out=vn, in0=w_v,
                                            scalar1=rr[:, 2 * NB + j:2 * NB + j + 1],
                                            scalar2=None, op0=ALU.mult)
                    an = an_pool.tile([P, DM], BF16)
                    nc.vector.tensor_scalar(out=an, in0=acx,
                                            scalar1=rna[:, j:j + 1],
                                            scalar2=None, op0=ALU.mult)

                    pt = psum_ct.tile([P, 4, P], BF16, tag="ct")
                    for k in range(2):
                        nc.tensor.transpose(pt[:, k, :], an[:, k * P:(k + 1) * P], id_sb)
                    for k in range(2):
                        nc.tensor.transpose(pt[:, 2 + k, :], vn[:, k * P:(k + 1) * P], id_sb)
                    xt = xt_pool.tile([P, KX, P], BF16)
                    nc.vector.tensor_copy(out=xt[:, 0:4, :], in_=pt)
                    nc.gpsimd.tensor_mul(out=xt[:, 4:6, :], in0=xt[:, 0:2, :],
                                         in1=xt[:, 2:4, :])

                    hh = hbuf.tile([P, HID], BF16, tag="hh")
                    for nh in range(2):
                        ph = psum_h.tile([P, 512], F32, tag="ph")
                        for k in range(KX):
                            nc.tensor.matmul(
                                ph, lhsT=xt[:, k, :],
                                rhs=w1_sb[:, k, nh * 512:(nh + 1) * 512],
                                start=(k == 0), stop=(k == KX - 1 and not b1_nz))
                        if b1_nz:
                            nc.tensor.matmul(ph, lhsT=ones_sb,
                                             rhs=b1_sb[:, nh * 512:(nh + 1) * 512],
                                             start=False, stop=True)
                        nc.scalar.activation(out=hh[:, nh * 512:(nh + 1) * 512],
                                             in_=ph, func=ACT.Gelu)
                    lsc = lsc_pool.tile([P, HID], BF16)
                    nc.vector.scalar_tensor_tensor(out=lsc, in0=hh, scalar=1.0,
                                                   in1=w2_sb, op0=ALU.mult,
                                                   op1=ALU.mult,
                                                   accum_out=lgb[:, j:j + 1])

                # ---- batched gate tail ----
                if b2f != 0.0:
                    nc.vector.tensor_scalar(out=lgb, in0=lgb, scalar1=float(b2f),
                                            scalar2=-12.0, op0=ALU.add, op1=ALU.max)
                    nc.vector.tensor_scalar(out=lgb, in0=lgb, scalar1=12.0,
                                            scalar2=None, op0=ALU.min)
                else:
                    nc.vector.tensor_scalar(out=lgb, in0=lgb, scalar1=-12.0,
                                            scalar2=12.0, op0=ALU.max, op1=ALU.min)
                nc.scalar.activation(out=ggc, in_=lgb, func=ACT.Tanh, scale=0.5)
                nc.vector.tensor_scalar(out=ggc, in0=ggc, scalar1=0.5,
                                        scalar2=0.5, op0=ALU.mult, op1=ALU.add)
                nc.vector.tensor_scalar(out=ggc, in0=ggc, scalar1=0.05,
                                        scalar2=0.95, op0=ALU.max, op1=ALU.min)
                # m2 = (1 - g) * rstd_v
                nc.vector.tensor_scalar(out=m2c, in0=ggc, scalar1=-1.0,
                                        scalar2=1.0, op0=ALU.mult, op1=ALU.add)
                nc.vector.tensor_tensor(out=m2c, in0=m2c, in1=rr[:, 0:NB],
                                        op=ALU.mult)

                # ---- fuse + output ----
                for j in range(NB):
                    t2 = t2_pool.tile([P, DM], BF16)
                    nc.vector.tensor_scalar(out=t2, in0=wv_t[j],
                                            scalar1=m2c[:, j:j + 1],
                                            scalar2=None, op0=ALU.mult)
                    o_sb = obuf.tile([P, DM], F32)
                    nc.vector.scalar_tensor_tensor(out=o_sb, in0=acx_t[j],
                                                   scalar=ggc[:, j:j + 1],
                                                   in1=t2, op0=ALU.mult,
                                                   op1=ALU.add)
                    i_glob = bidx * NB + j
                    nc.sync.dma_start(out=out.ap()[i_glob * P:(i_glob + 1) * P, :],
                                      in_=o_sb)

    nc.compile()
    return nc


def _prepare_in_maps(video_seq, audio_seq, Wv, bv, Wa, ba, theta, W1, b1, W2, b2):
    video_seq = np.asarray(video_seq, np.float32)
    audio_seq = np.asarray(audio_seq, np.float32)
    th = float(np.clip(np.float32(theta), -12.0, 12.0))
    delta = 2.0 + 4.0 / (1.0 + np.exp(-th))
    cmats = _build_cmats(float(delta))

    Wv = np.asarray(Wv, np.float32)
    Wa = np.asarray(Wa, np.float32)
    bv = np.asarray(bv, np.float32)
    ba = np.asarray(ba, np.float32)
    # pre-center output features: mean over DM commutes through the matmul
    Wv_c = Wv - Wv.mean(axis=1, keepdims=True)
    Wa_c = Wa - Wa.mean(axis=1, keepdims=True)
    bv_c = bv - bv.mean()
    ba_c = ba - ba.mean()

    bv_nz = bool(np.any(bv_c != 0))
    ba_nz = bool(np.any(ba_c != 0))
    b1_nz = bool(np.any(np.asarray(b1) != 0))
    b2f = float(np.asarray(b2).reshape(-1)[0])

    shared = {
        "wv": np.ascontiguousarray(Wv_c),
        "wa": np.ascontiguousarray(Wa_c),
        "w1": np.ascontiguousarray(
            np.asarray(W1, np.float32).astype(ml_dtypes.bfloat16)),
        "w2h": np.ascontiguousarray(
            np.asarray(W2, np.float32).reshape(HID).astype(ml_dtypes.bfloat16)),
        "cm": np.ascontiguousarray(cmats.astype(ml_dtypes.bfloat16)),
        "ident": np.eye(P, dtype=np.float32).astype(ml_dtypes.bfloat16),
    }
    if bv_nz:
        shared["bvr"] = np.ascontiguousarray(bv_c.reshape(1, DM))
    if ba_nz:
        shared["bar"] = np.ascontiguousarray(ba_c.reshape(1, DM))
    if b1_nz:
        shared["b1r"] = np.ascontiguousarray(np.asarray(b1, np.float32).reshape(1, HID))

    in_maps = []
    for b in range(B):
        m = dict(shared)
        m["vT"] = np.ascontiguousarray(video_seq[b].T)
        m["aT"] = np.ascontiguousarray(audio_seq[b].T)
        in_maps.append(m)
    return in_maps, (bv_nz, ba_nz, b1_nz, b2f)


def kernel(video_seq, audio_seq, Wv, bv, Wa, ba, theta, W1, b1, W2, b2):
    in_maps, key = _prepare_in_maps(video_seq, audio_seq, Wv, bv, Wa, ba,
                                    theta, W1, b1, W2, b2)
    if key not in _nc_cache:
        _nc_cache[key] = _build(*key)
    nc = _nc_cache[key]
    res = run_bass_kernel_spmd(nc, in_maps, list(range(B)))
    return np.stack([res.results[i]["out"] for i in range(B)])
